# revision 1
# baseline (speedup 1.0000x reference)
"""Expert-parallel MoE (top-1 routing) kernel for 8 TRN2 NeuronCores.

Strategy (per the expert-parallel sharding hint): the 8 experts are sharded
1:1 across the 8 cores. The router is a 0.1%-of-FLOPs linear; it is computed
host-side in float64 to decide the token->expert dispatch (the all-to-all is
realized as the host->device sharding itself: each token's activations are
DMA'd only to the core owning its expert). Each core then runs the dense
expert MLP  y = (silu(x @ gw.T) * (x @ up.T)) @ dw.T  over its gathered
tokens (padded to a uniform capacity C) in bf16 with fp32 PSUM accumulation.

Layout: everything on device is kept "activation-transposed" so all three
matmuls contract over the partition dimension with zero on-device transposes:
  g_T[i_tile] = sum_k gwT[k, i].T @ x_T[k]      (psum [128(I), C])
  a_T = silu(g_T) * u_T                          (sbuf bf16)
  y_T[m_tile] += dwT[i, m].T @ a_T[i]            (psum [128(H), C], 22-step acc)
Weights are pre-transposed + bf16-cast host-side and packed per i-tile
(gate|up|down) so each iteration issues ONE contiguous 768 KiB DMA.

Raw bass (no Tile scheduler): Tile's sem assignment overflows walrus's
per-instruction sync-wait encoding limits on this kernel; explicit
per-engine streams with standalone wait_ge instructions avoid that
entirely and give a deterministic software pipeline:
  SP   : x DMA, 22 weight DMAs, 8 output DMAs (gated on DVE copies)
  PE   : per i: 8 g-matmuls, 8 u-matmuls (gated on w DMA i), 8 y-matmuls
         (gated on a_T[i] from DVE)
  ACT  : per i: silu(g)->sbuf, copy(u)->sbuf   (gated on PE psum stops)
  DVE  : per i: a_T[i] = sg*us (bf16)          (gated on ACT)
         tail: 8 psum->sbuf y copies           (gated on last PE matmul)
"""

import numpy as np
import ml_dtypes
from contextlib import ExitStack

import concourse.bass as bass
import concourse.mybir as mybir
from concourse.bass_utils import run_bass_kernel_spmd

S, B, H, I, E = 512, 2, 1024, 2816, 8
KT, IT, MT = H // 128, I // 128, H // 128  # 8, 22, 8
_BF = mybir.dt.bfloat16
_F32 = mybir.dt.float32

_nc_cache: dict = {}


def _build(C: int) -> bass.Bass:
    """One-core program; SPMD across 8 cores (same shapes, per-core data)."""
    nc = bass.Bass()
    xt = nc.dram_tensor("xt", [128, KT * C], _BF, kind="ExternalInput")
    # packed weights per i-tile: [gate (KT*128) | up (KT*128) | down (MT*128)]
    wt = nc.dram_tensor("wt", [IT, 128, 3 * KT * 128], _BF, kind="ExternalInput")
    yt = nc.dram_tensor("yt", [128, MT * C], _F32, kind="ExternalOutput")

    assert 2 * C <= 512, "two y slices must fit one PSUM bank"
    GW0, UW0, DW0 = 0, KT * 128, 2 * KT * 128
    W = 3 * KT * 128  # 3072 cols per i-tile

    with ExitStack() as ctx:
        x_sb = ctx.enter_context(nc.sbuf_tensor([128, KT * C], _BF))
        w_sb = ctx.enter_context(nc.sbuf_tensor([128, IT * W], _BF))
        sg_sb = ctx.enter_context(nc.sbuf_tensor([128, IT * C], _F32))
        us_sb = ctx.enter_context(nc.sbuf_tensor([128, IT * C], _F32))
        a_sb = ctx.enter_context(nc.sbuf_tensor([128, IT * C], _BF))
        y_sb = ctx.enter_context(nc.sbuf_tensor([128, MT * C], _F32))
        # every PSUM tensor is one full 2 KiB bank ([128, 512] f32): matmul
        # outputs must not cross bank boundaries, and the bump allocator
        # would otherwise pack tensors across banks
        g_ps = [
            ctx.enter_context(nc.psum_tensor(f"g_ps{j}", [128, 512], _F32))
            for j in range(2)
        ]
        u_ps = [
            ctx.enter_context(nc.psum_tensor(f"u_ps{j}", [128, 512], _F32))
            for j in range(2)
        ]
        y_ps = [
            ctx.enter_context(nc.psum_tensor(f"y_ps{j}", [128, 512], _F32))
            for j in range(4)
        ]

        def yslice(m):
            return y_ps[m // 2][:, (m % 2) * 256 : (m % 2) * 256 + C]

        x_sem = ctx.enter_context(nc.semaphore())
        w_sem = [ctx.enter_context(nc.semaphore(name=f"w_sem{j}")) for j in range(IT)]
        pe_g = ctx.enter_context(nc.semaphore())
        pe_u = ctx.enter_context(nc.semaphore())
        pe_done = ctx.enter_context(nc.semaphore())
        act_sem = ctx.enter_context(nc.semaphore())
        dve_sem = ctx.enter_context(nc.semaphore())
        dma_sem = ctx.enter_context(nc.semaphore())
        block = ctx.enter_context(nc.Block())

        @block.sync
        def _(sync):
            nc.sync.dma_start(x_sb[:], xt[:]).then_inc(x_sem, 16)
            for i in range(IT):
                nc.sync.dma_start(
                    w_sb[:, i * W : (i + 1) * W], wt[i]
                ).then_inc(w_sem[i], 16)
            for m in range(MT):
                # copy m done once dve_sem reaches IT (muls) + m+1 (copies)
                nc.sync.wait_ge(dve_sem, IT + m + 1)
                nc.sync.dma_start(
                    yt[:, m * C : (m + 1) * C], y_sb[:, m * C : (m + 1) * C]
                ).then_inc(dma_sem, 16)
            nc.sync.wait_ge(dma_sem, MT * 16)

        @block.tensor
        def _(tensor):
            nc.tensor.wait_ge(x_sem, 16)
            for i in range(IT):
                pp = i % 2
                nc.tensor.wait_ge(w_sem[i], 16)
                if i >= 2:
                    # ACT must have drained g_ps/u_ps of i-2 (2 ACT ops per i)
                    nc.tensor.wait_ge(act_sem, 2 * (i - 1))
                for k in range(KT):
                    mm = nc.tensor.matmul(
                        g_ps[pp][:, :C],
                        w_sb[:, i * W + GW0 + k * 128 : i * W + GW0 + (k + 1) * 128],
                        x_sb[:, k * C : (k + 1) * C],
                        start=(k == 0),
                        stop=(k == KT - 1),
                    )
                mm.then_inc(pe_g, 1)
                for k in range(KT):
                    mm = nc.tensor.matmul(
                        u_ps[pp][:, :C],
                        w_sb[:, i * W + UW0 + k * 128 : i * W + UW0 + (k + 1) * 128],
                        x_sb[:, k * C : (k + 1) * C],
                        start=(k == 0),
                        stop=(k == KT - 1),
                    )
                mm.then_inc(pe_u, 1)
                nc.tensor.wait_ge(dve_sem, i + 1)  # a_T[i] ready
                for m in range(MT):
                    # start=True clears has_written for the WHOLE psum bank,
                    # so only the first (even) slice of each bank may set it;
                    # the odd slice's first write then lands on cleared
                    # has_written and overwrites cleanly.
                    mm = nc.tensor.matmul(
                        yslice(m),
                        w_sb[:, i * W + DW0 + m * 128 : i * W + DW0 + (m + 1) * 128],
                        a_sb[:, i * C : (i + 1) * C],
                        start=(i == 0 and m % 2 == 0),
                        stop=(i == IT - 1),
                        skip_group_check=True,
                    )
                if i == IT - 1:
                    mm.then_inc(pe_done, 1)

        @block.scalar
        def _(scalar):
            for i in range(IT):
                pp = i % 2
                nc.scalar.wait_ge(pe_g, i + 1)
                nc.scalar.activation(
                    sg_sb[:, i * C : (i + 1) * C],
                    g_ps[pp][:, :C],
                    mybir.ActivationFunctionType.Silu,
                ).then_inc(act_sem, 1)
                nc.scalar.wait_ge(pe_u, i + 1)
                nc.scalar.copy(
                    us_sb[:, i * C : (i + 1) * C], u_ps[pp][:, :C]
                ).then_inc(act_sem, 1)

        @block.vector
        def _(vector):
            for i in range(IT):
                nc.vector.wait_ge(act_sem, 2 * i + 2)
                nc.vector.tensor_mul(
                    a_sb[:, i * C : (i + 1) * C],
                    sg_sb[:, i * C : (i + 1) * C],
                    us_sb[:, i * C : (i + 1) * C],
                ).then_inc(dve_sem, 1)
            nc.vector.wait_ge(pe_done, 1)
            for m in range(MT):
                nc.vector.tensor_copy(
                    y_sb[:, m * C : (m + 1) * C], yslice(m)
                ).then_inc(dve_sem, 1)

    return nc


def _bf(x):
    return np.ascontiguousarray(x).astype(ml_dtypes.bfloat16)


def run(hidden_states, router_w, gate_w, up_w, down_w, trace=False):
    h = np.asarray(hidden_states, dtype=np.float32)
    rw = np.asarray(router_w, dtype=np.float32)
    gw = np.asarray(gate_w, dtype=np.float32)
    uw = np.asarray(up_w, dtype=np.float32)
    dw = np.asarray(down_w, dtype=np.float32)

    T = S * B
    hf = h.reshape(T, H)
    logits = hf.astype(np.float64) @ rw.astype(np.float64).T
    ids = logits.argmax(-1)
    idx = [np.where(ids == e)[0] for e in range(E)]
    maxc = max(len(s) for s in idx)
    C = max(128, -(-maxc // 32) * 32)

    if C not in _nc_cache:
        _nc_cache[C] = _build(C)
    nc = _nc_cache[C]

    in_maps = []
    for e in range(E):
        sel = idx[e]
        xp = np.zeros((C, H), np.float32)
        xp[: len(sel)] = hf[sel]
        # xt[p, k*C+c] = x[c, k*128+p]
        xt = _bf(xp.reshape(C, KT, 128).transpose(2, 1, 0).reshape(128, KT * C))
        # gwt[i, p, k*128+m] = gate_w[e][i*128+m, k*128+p]
        gwt = gw[e].reshape(IT, 128, KT, 128).transpose(0, 3, 2, 1).reshape(IT, 128, KT * 128)
        uwt = uw[e].reshape(IT, 128, KT, 128).transpose(0, 3, 2, 1).reshape(IT, 128, KT * 128)
        # dwt[i, p, m*128+mm] = down_w[e][m*128+mm, i*128+p]
        dwt = dw[e].reshape(MT, 128, IT, 128).transpose(2, 3, 0, 1).reshape(IT, 128, MT * 128)
        wtv = _bf(np.concatenate([gwt, uwt, dwt], axis=2))
        in_maps.append({"xt": xt, "wt": wtv})

    res = run_bass_kernel_spmd(nc, in_maps, core_ids=list(range(E)), trace=trace)

    out = np.zeros((T, H), np.float32)
    for e in range(E):
        ytv = np.asarray(res.results[e]["yt"], dtype=np.float32)
        # y[c, m*128+p] = yt[p, m*C+c]
        y = ytv.reshape(128, MT, C).transpose(2, 1, 0).reshape(C, H)
        out[idx[e]] = y[: len(idx[e])]
    return out.reshape(S, B, H), res


def kernel(**inputs) -> np.ndarray:
    out, _ = run(**inputs)
    return out



# revision 2
# speedup vs baseline: 1.3533x; 1.3533x over previous
"""Expert-parallel MoE (top-1 routing) kernel for 8 TRN2 NeuronCores.

Strategy (per the expert-parallel sharding hint): the 8 experts are sharded
1:1 across the 8 cores. The router is a 0.1%-of-FLOPs linear; it is computed
host-side in float64 to decide the token->expert dispatch (the all-to-all is
realized as the host->device sharding itself: each token's activations are
DMA'd only to the core owning its expert). Each core then runs the dense
expert MLP  y = (silu(x @ gw.T) * (x @ up.T)) @ dw.T  over its gathered
tokens (padded to a uniform capacity C) in bf16 with fp32 PSUM accumulation.

Layout: everything on device is kept "activation-transposed" so all three
matmuls contract over the partition dimension with zero on-device transposes:
  g_T[i_tile] = sum_k gwT[k, i].T @ x_T[k]      (psum [128(I), C])
  a_T = silu(g_T) * u_T                          (sbuf bf16)
  y_T[m_tile] += dwT[i, m].T @ a_T[i]            (psum [128(H), C], 22-step acc)
Weights are pre-transposed + bf16-cast host-side and packed per i-tile
(gate|up|down) so each iteration issues ONE contiguous 768 KiB DMA whose
128 6-KiB descriptors spray across all 16 DMA queues (~2.0 us/tile at the
~400 GB/s aggregate HBM read rate).

PE-warmth-critical schedule (raw bass, explicit per-engine streams): the
down-projection matmuls LAG ONE i-TILE behind the gate/up matmuls, so the
tensor engine never waits mid-stream on the silu->mul chain of the SAME
iteration. Without the lag the PE idles ~1.3 us every i-tile and the HAM
clock gate keeps it throttled at 1.2 GHz (133 ns/MM at C=160) for the whole
kernel; with a continuous MM stream it un-throttles to 2.4 GHz (~65 ns/MM)
and the kernel becomes weight-DMA-bound (~17.3 MB bf16 per core).

  SP   : w0 DMA, x DMA, w1..w21 DMAs, 8 output DMAs (gated on DVE copies)
  PE   : per i: 8 g-matmuls, 8 u-matmuls (gated on w DMA i), then 8
         y-matmuls OF TILE i-1 (gated on a_T[i-1] from DVE — ready long ago)
  ACT  : per i: silu(g)->sbuf, copy(u)->sbuf   (gated on PE psum stops)
  DVE  : per i: a_T[i] = sg*us (bf16)          (gated on ACT)
         tail: 8 psum->sbuf y copies           (gated on last PE matmul)
"""

import numpy as np
import ml_dtypes
from contextlib import ExitStack

import concourse.bass as bass
import concourse.mybir as mybir
from concourse.bass_utils import run_bass_kernel_spmd

S, B, H, I, E = 512, 2, 1024, 2816, 8
KT, IT, MT = H // 128, I // 128, H // 128  # 8, 22, 8
_BF = mybir.dt.bfloat16
_F32 = mybir.dt.float32

_nc_cache: dict = {}


def _build(C: int) -> bass.Bass:
    """One-core program; SPMD across 8 cores (same shapes, per-core data)."""
    nc = bass.Bass()
    xt = nc.dram_tensor("xt", [128, KT * C], _BF, kind="ExternalInput")
    # packed weights per i-tile: [gate (KT*128) | up (KT*128) | down (MT*128)]
    wt = nc.dram_tensor("wt", [IT, 128, 3 * KT * 128], _BF, kind="ExternalInput")
    yt = nc.dram_tensor("yt", [128, MT * C], _F32, kind="ExternalOutput")

    assert 2 * C <= 512, "two y slices must fit one PSUM bank"
    GW0, UW0, DW0 = 0, KT * 128, 2 * KT * 128
    W = 3 * KT * 128  # 3072 cols per i-tile

    with ExitStack() as ctx:
        x_sb = ctx.enter_context(nc.sbuf_tensor([128, KT * C], _BF))
        w_sb = ctx.enter_context(nc.sbuf_tensor([128, IT * W], _BF))
        sg_sb = ctx.enter_context(nc.sbuf_tensor([128, IT * C], _F32))
        us_sb = ctx.enter_context(nc.sbuf_tensor([128, IT * C], _F32))
        a_sb = ctx.enter_context(nc.sbuf_tensor([128, IT * C], _BF))
        y_sb = ctx.enter_context(nc.sbuf_tensor([128, MT * C], _F32))
        # every PSUM tensor is one full 2 KiB bank ([128, 512] f32): matmul
        # outputs must not cross bank boundaries, and the bump allocator
        # would otherwise pack tensors across banks
        g_ps = [
            ctx.enter_context(nc.psum_tensor(f"g_ps{j}", [128, 512], _F32))
            for j in range(2)
        ]
        u_ps = [
            ctx.enter_context(nc.psum_tensor(f"u_ps{j}", [128, 512], _F32))
            for j in range(2)
        ]
        y_ps = [
            ctx.enter_context(nc.psum_tensor(f"y_ps{j}", [128, 512], _F32))
            for j in range(4)
        ]

        def yslice(m):
            return y_ps[m // 2][:, (m % 2) * 256 : (m % 2) * 256 + C]

        x_sem = ctx.enter_context(nc.semaphore())
        w_sem = [ctx.enter_context(nc.semaphore(name=f"w_sem{j}")) for j in range(IT)]
        pe_g = ctx.enter_context(nc.semaphore())
        pe_u = ctx.enter_context(nc.semaphore())
        pe_done = ctx.enter_context(nc.semaphore())
        act_sem = ctx.enter_context(nc.semaphore())
        dve_sem = ctx.enter_context(nc.semaphore())
        dma_sem = ctx.enter_context(nc.semaphore())
        block = ctx.enter_context(nc.Block())

        @block.sync
        def _(sync):
            # w0 first: the PE's first dependency; x is smaller and second
            nc.sync.dma_start(w_sb[:, 0:W], wt[0]).then_inc(w_sem[0], 16)
            nc.sync.dma_start(x_sb[:], xt[:]).then_inc(x_sem, 16)
            for i in range(1, IT):
                nc.sync.dma_start(
                    w_sb[:, i * W : (i + 1) * W], wt[i]
                ).then_inc(w_sem[i], 16)
            for m in range(MT):
                # copy m done once dve_sem reaches IT (muls) + m+1 (copies)
                nc.sync.wait_ge(dve_sem, IT + m + 1)
                nc.sync.dma_start(
                    yt[:, m * C : (m + 1) * C], y_sb[:, m * C : (m + 1) * C]
                ).then_inc(dma_sem, 16)
            nc.sync.wait_ge(dma_sem, MT * 16)

        @block.tensor
        def _(tensor):
            def y_group(j):
                # down-projection matmuls of i-tile j (issued during i=j+1)
                nc.tensor.wait_ge(dve_sem, j + 1)  # a_T[j] ready
                for m in range(MT):
                    # start=True clears has_written for the WHOLE psum bank,
                    # so only the first (even) slice of each bank may set it;
                    # the odd slice's first write then lands on cleared
                    # has_written and overwrites cleanly.
                    mm = nc.tensor.matmul(
                        yslice(m),
                        w_sb[:, j * W + DW0 + m * 128 : j * W + DW0 + (m + 1) * 128],
                        a_sb[:, j * C : (j + 1) * C],
                        start=(j == 0 and m % 2 == 0),
                        stop=(j == IT - 1),
                        skip_group_check=True,
                    )
                return mm

            nc.tensor.wait_ge(x_sem, 16)
            for i in range(IT):
                pp = i % 2
                nc.tensor.wait_ge(w_sem[i], 16)
                if i >= 2:
                    # ACT must have drained g_ps/u_ps of i-2 (2 ACT ops per i)
                    nc.tensor.wait_ge(act_sem, 2 * (i - 1))
                for k in range(KT):
                    mm = nc.tensor.matmul(
                        g_ps[pp][:, :C],
                        w_sb[:, i * W + GW0 + k * 128 : i * W + GW0 + (k + 1) * 128],
                        x_sb[:, k * C : (k + 1) * C],
                        start=(k == 0),
                        stop=(k == KT - 1),
                    )
                mm.then_inc(pe_g, 1)
                for k in range(KT):
                    mm = nc.tensor.matmul(
                        u_ps[pp][:, :C],
                        w_sb[:, i * W + UW0 + k * 128 : i * W + UW0 + (k + 1) * 128],
                        x_sb[:, k * C : (k + 1) * C],
                        start=(k == 0),
                        stop=(k == KT - 1),
                    )
                mm.then_inc(pe_u, 1)
                if i >= 1:
                    y_group(i - 1)
            y_group(IT - 1).then_inc(pe_done, 1)

        @block.scalar
        def _(scalar):
            for i in range(IT):
                pp = i % 2
                nc.scalar.wait_ge(pe_g, i + 1)
                nc.scalar.activation(
                    sg_sb[:, i * C : (i + 1) * C],
                    g_ps[pp][:, :C],
                    mybir.ActivationFunctionType.Silu,
                ).then_inc(act_sem, 1)
                nc.scalar.wait_ge(pe_u, i + 1)
                nc.scalar.copy(
                    us_sb[:, i * C : (i + 1) * C], u_ps[pp][:, :C]
                ).then_inc(act_sem, 1)

        @block.vector
        def _(vector):
            for i in range(IT):
                nc.vector.wait_ge(act_sem, 2 * i + 2)
                nc.vector.tensor_mul(
                    a_sb[:, i * C : (i + 1) * C],
                    sg_sb[:, i * C : (i + 1) * C],
                    us_sb[:, i * C : (i + 1) * C],
                ).then_inc(dve_sem, 1)
            nc.vector.wait_ge(pe_done, 1)
            for m in range(MT):
                nc.vector.tensor_copy(
                    y_sb[:, m * C : (m + 1) * C], yslice(m)
                ).then_inc(dve_sem, 1)

    return nc


def _bf(x):
    return np.ascontiguousarray(x).astype(ml_dtypes.bfloat16)


def run(hidden_states, router_w, gate_w, up_w, down_w, trace=False):
    h = np.asarray(hidden_states, dtype=np.float32)
    rw = np.asarray(router_w, dtype=np.float32)
    gw = np.asarray(gate_w, dtype=np.float32)
    uw = np.asarray(up_w, dtype=np.float32)
    dw = np.asarray(down_w, dtype=np.float32)

    T = S * B
    hf = h.reshape(T, H)
    logits = hf.astype(np.float64) @ rw.astype(np.float64).T
    ids = logits.argmax(-1)
    idx = [np.where(ids == e)[0] for e in range(E)]
    maxc = max(len(s) for s in idx)
    C = max(128, -(-maxc // 16) * 16)

    if C not in _nc_cache:
        _nc_cache[C] = _build(C)
    nc = _nc_cache[C]

    in_maps = []
    for e in range(E):
        sel = idx[e]
        xp = np.zeros((C, H), np.float32)
        xp[: len(sel)] = hf[sel]
        # xt[p, k*C+c] = x[c, k*128+p]
        xt = _bf(xp.reshape(C, KT, 128).transpose(2, 1, 0).reshape(128, KT * C))
        # gwt[i, p, k*128+m] = gate_w[e][i*128+m, k*128+p]
        gwt = gw[e].reshape(IT, 128, KT, 128).transpose(0, 3, 2, 1).reshape(IT, 128, KT * 128)
        uwt = uw[e].reshape(IT, 128, KT, 128).transpose(0, 3, 2, 1).reshape(IT, 128, KT * 128)
        # dwt[i, p, m*128+mm] = down_w[e][m*128+mm, i*128+p]
        dwt = dw[e].reshape(MT, 128, IT, 128).transpose(2, 3, 0, 1).reshape(IT, 128, MT * 128)
        wtv = _bf(np.concatenate([gwt, uwt, dwt], axis=2))
        in_maps.append({"xt": xt, "wt": wtv})

    res = run_bass_kernel_spmd(nc, in_maps, core_ids=list(range(E)), trace=trace)

    out = np.zeros((T, H), np.float32)
    for e in range(E):
        ytv = np.asarray(res.results[e]["yt"], dtype=np.float32)
        # y[c, m*128+p] = yt[p, m*C+c]
        y = ytv.reshape(128, MT, C).transpose(2, 1, 0).reshape(C, H)
        out[idx[e]] = y[: len(idx[e])]
    return out.reshape(S, B, H), res


def kernel(**inputs) -> np.ndarray:
    out, _ = run(**inputs)
    return out


# revision 7
# speedup vs baseline: 1.4172x; 1.0472x over previous
"""Expert-parallel MoE (top-1 routing) kernel for 8 TRN2 NeuronCores.

Strategy (per the expert-parallel sharding hint): the 8 experts are sharded
1:1 across the 8 cores. The router is a 0.1%-of-FLOPs linear; it is computed
host-side in float64 to decide the token->expert dispatch (the all-to-all is
realized as the host->device sharding itself: each token's activations are
DMA'd only to the core owning its expert). Each core then runs the dense
expert MLP  y = (silu(x @ gw.T) * (x @ up.T)) @ dw.T  over its gathered
tokens (padded to a uniform capacity C) in bf16 with fp32 PSUM accumulation.

Layout: everything on device is kept "activation-transposed" so all three
matmuls contract over the partition dimension with zero on-device transposes:
  g_T[i_tile] = sum_k gwT[k, i].T @ x_T[k]      (psum [128(I), C])
  a_T = silu(g_T) * u_T                          (sbuf bf16)
  y_T[m_tile] += dwT[i, m].T @ a_T[i]            (psum [128(H), C], 22-step acc)
Weights are pre-transposed + bf16-cast host-side and packed per i-tile
(gate|up|down) so each iteration issues ONE contiguous 768 KiB DMA whose
128 6-KiB descriptors spray across all 16 DMA queues (~2.0 us/tile at the
~400 GB/s aggregate HBM read rate).

PE-warmth-critical schedule (raw bass, explicit per-engine streams): the
down-projection matmuls LAG ONE i-TILE behind the gate/up matmuls, so the
tensor engine never waits mid-stream on the silu->mul chain of the SAME
iteration. Without the lag the PE idles ~1.3 us every i-tile and the HAM
clock gate keeps it throttled at 1.2 GHz (133 ns/MM at C=160) for the whole
kernel; with a continuous MM stream it un-throttles to 2.4 GHz (~65 ns/MM)
and the kernel becomes weight-DMA-bound (~17.3 MB bf16 per core).

  SP   : w0 DMA, x DMA, w1..w21 DMAs, 8 output DMAs (gated on DVE copies)
  PE   : per i: 8 g-matmuls, 8 u-matmuls (gated on w DMA i), then 8
         y-matmuls OF TILE i-1 (gated on a_T[i-1] from DVE — ready long ago)
  ACT  : per i: silu(g)->sbuf, copy(u)->sbuf   (gated on PE psum stops)
  DVE  : per i: a_T[i] = sg*us (bf16)          (gated on ACT)
         tail: 8 psum->sbuf y copies           (gated on last PE matmul)
"""

import numpy as np
import ml_dtypes
from contextlib import ExitStack

import concourse.bass as bass
import concourse.mybir as mybir
from concourse.bass_utils import run_bass_kernel_spmd

S, B, H, I, E = 512, 2, 1024, 2816, 8
KT, IT, MT = H // 128, I // 128, H // 128  # 8, 22, 8
_BF = mybir.dt.bfloat16
_F32 = mybir.dt.float32

_nc_cache: dict = {}


def _build(C: int) -> bass.Bass:
    """One-core program; SPMD across 8 cores (same shapes, per-core data)."""
    nc = bass.Bass()
    xt = nc.dram_tensor("xt", [128, KT * C], _BF, kind="ExternalInput")
    # packed weights per i-tile: [gate (KT*128) | up (KT*128) | down (MT*128)]
    wt = nc.dram_tensor("wt", [IT, 128, 3 * KT * 128], _BF, kind="ExternalInput")
    yt = nc.dram_tensor("yt", [128, MT * C], _BF, kind="ExternalOutput")

    assert 2 * C <= 512, "two y slices must fit one PSUM bank"
    GW0, UW0, DW0 = 0, KT * 128, 2 * KT * 128
    W = 3 * KT * 128  # 3072 cols per i-tile

    with ExitStack() as ctx:
        x_sb = ctx.enter_context(nc.sbuf_tensor([128, KT * C], _BF))
        w_sb = ctx.enter_context(nc.sbuf_tensor([128, IT * W], _BF))
        sg_sb = ctx.enter_context(nc.sbuf_tensor([128, IT * C], _F32))
        us_sb = ctx.enter_context(nc.sbuf_tensor([128, IT * C], _F32))
        a_sb = ctx.enter_context(nc.sbuf_tensor([128, IT * C], _BF))
        y_sb = ctx.enter_context(nc.sbuf_tensor([128, MT * C], _BF))
        # every PSUM tensor is one full 2 KiB bank ([128, 512] f32): matmul
        # outputs must not cross bank boundaries, and the bump allocator
        # would otherwise pack tensors across banks
        g_ps = [
            ctx.enter_context(nc.psum_tensor(f"g_ps{j}", [128, 512], _F32))
            for j in range(2)
        ]
        u_ps = [
            ctx.enter_context(nc.psum_tensor(f"u_ps{j}", [128, 512], _F32))
            for j in range(2)
        ]
        y_ps = [
            ctx.enter_context(nc.psum_tensor(f"y_ps{j}", [128, 512], _F32))
            for j in range(4)
        ]

        def yslice(m):
            return y_ps[m // 2][:, (m % 2) * 256 : (m % 2) * 256 + C]

        x_sem = ctx.enter_context(nc.semaphore())
        w_sem = [ctx.enter_context(nc.semaphore(name=f"w_sem{j}")) for j in range(IT)]
        pe_g = ctx.enter_context(nc.semaphore())
        pe_u = ctx.enter_context(nc.semaphore())
        pe_done = ctx.enter_context(nc.semaphore())
        act_sem = ctx.enter_context(nc.semaphore())
        dve_sem = ctx.enter_context(nc.semaphore())
        dma_sem = ctx.enter_context(nc.semaphore())
        block = ctx.enter_context(nc.Block())

        @block.sync
        def _(sync):
            # x first (drains fast), then weights in consumption order
            nc.sync.dma_start(x_sb[:], xt[:]).then_inc(x_sem, 16)
            for i in range(IT):
                nc.sync.dma_start(
                    w_sb[:, i * W : (i + 1) * W], wt[i]
                ).then_inc(w_sem[i], 16)
            # one merged output DMA once all 4 bank copies are done
            nc.sync.wait_ge(dve_sem, IT + 4)
            nc.sync.dma_start(yt[:], y_sb[:]).then_inc(dma_sem, 16)
            nc.sync.wait_ge(dma_sem, 16)

        @block.tensor
        def _(tensor):
            def y_group(j):
                # down-projection matmuls of i-tile j (issued during i=j+1)
                nc.tensor.wait_ge(dve_sem, j + 1)  # a_T[j] ready
                for m in range(MT):
                    # start=True clears has_written for the WHOLE psum bank,
                    # so only the first (even) slice of each bank may set it;
                    # the odd slice's first write then lands on cleared
                    # has_written and overwrites cleanly.
                    mm = nc.tensor.matmul(
                        yslice(m),
                        w_sb[:, j * W + DW0 + m * 128 : j * W + DW0 + (m + 1) * 128],
                        a_sb[:, j * C : (j + 1) * C],
                        start=(j == 0 and m % 2 == 0),
                        stop=(j == IT - 1),
                        skip_group_check=True,
                    )
                return mm

            nc.tensor.wait_ge(x_sem, 16)
            for i in range(IT):
                pp = i % 2
                nc.tensor.wait_ge(w_sem[i], 16)
                if i >= 2:
                    # ACT must have drained g_ps/u_ps of i-2 (2 ACT ops per i)
                    nc.tensor.wait_ge(act_sem, 2 * (i - 1))
                for k in range(KT):
                    mm = nc.tensor.matmul(
                        g_ps[pp][:, :C],
                        w_sb[:, i * W + GW0 + k * 128 : i * W + GW0 + (k + 1) * 128],
                        x_sb[:, k * C : (k + 1) * C],
                        start=(k == 0),
                        stop=(k == KT - 1),
                    )
                mm.then_inc(pe_g, 1)
                for k in range(KT):
                    mm = nc.tensor.matmul(
                        u_ps[pp][:, :C],
                        w_sb[:, i * W + UW0 + k * 128 : i * W + UW0 + (k + 1) * 128],
                        x_sb[:, k * C : (k + 1) * C],
                        start=(k == 0),
                        stop=(k == KT - 1),
                    )
                mm.then_inc(pe_u, 1)
                if i >= 1:
                    y_group(i - 1)
            y_group(IT - 1).then_inc(pe_done, 1)

        @block.scalar
        def _(scalar):
            for i in range(IT):
                pp = i % 2
                nc.scalar.wait_ge(pe_g, i + 1)
                nc.scalar.activation(
                    sg_sb[:, i * C : (i + 1) * C],
                    g_ps[pp][:, :C],
                    mybir.ActivationFunctionType.Silu,
                ).then_inc(act_sem, 1)
                nc.scalar.wait_ge(pe_u, i + 1)
                nc.scalar.copy(
                    us_sb[:, i * C : (i + 1) * C], u_ps[pp][:, :C]
                ).then_inc(act_sem, 1)

        @block.vector
        def _(vector):
            for i in range(IT):
                nc.vector.wait_ge(act_sem, 2 * i + 2)
                nc.vector.tensor_mul(
                    a_sb[:, i * C : (i + 1) * C],
                    sg_sb[:, i * C : (i + 1) * C],
                    us_sb[:, i * C : (i + 1) * C],
                ).then_inc(dve_sem, 1)
            nc.vector.wait_ge(pe_done, 1)
            for j in range(4):
                # one copy per psum bank: both 256-aligned y slices at once,
                # f32 psum -> bf16 sbuf cast in the DVE
                src = y_ps[j].rearrange("p (s c) -> p s c", s=2)[:, :, :C]
                dst = y_sb[:, 2 * j * C : (2 * j + 2) * C].rearrange(
                    "p (s c) -> p s c", s=2
                )
                nc.vector.tensor_copy(dst, src).then_inc(dve_sem, 1)

    return nc


def _bf(x):
    return np.ascontiguousarray(x).astype(ml_dtypes.bfloat16)


def run(hidden_states, router_w, gate_w, up_w, down_w, trace=False):
    h = np.asarray(hidden_states, dtype=np.float32)
    rw = np.asarray(router_w, dtype=np.float32)
    gw = np.asarray(gate_w, dtype=np.float32)
    uw = np.asarray(up_w, dtype=np.float32)
    dw = np.asarray(down_w, dtype=np.float32)

    T = S * B
    hf = h.reshape(T, H)
    logits = hf.astype(np.float64) @ rw.astype(np.float64).T
    ids = logits.argmax(-1)
    idx = [np.where(ids == e)[0] for e in range(E)]
    maxc = max(len(s) for s in idx)
    C = max(128, -(-maxc // 16) * 16)

    if C not in _nc_cache:
        _nc_cache[C] = _build(C)
    nc = _nc_cache[C]

    in_maps = []
    for e in range(E):
        sel = idx[e]
        xp = np.zeros((C, H), np.float32)
        xp[: len(sel)] = hf[sel]
        # xt[p, k*C+c] = x[c, k*128+p]
        xt = _bf(xp.reshape(C, KT, 128).transpose(2, 1, 0).reshape(128, KT * C))
        # gwt[i, p, k*128+m] = gate_w[e][i*128+m, k*128+p]
        gwt = gw[e].reshape(IT, 128, KT, 128).transpose(0, 3, 2, 1).reshape(IT, 128, KT * 128)
        uwt = uw[e].reshape(IT, 128, KT, 128).transpose(0, 3, 2, 1).reshape(IT, 128, KT * 128)
        # dwt[i, p, m*128+mm] = down_w[e][m*128+mm, i*128+p]
        dwt = dw[e].reshape(MT, 128, IT, 128).transpose(2, 3, 0, 1).reshape(IT, 128, MT * 128)
        wtv = _bf(np.concatenate([gwt, uwt, dwt], axis=2))
        in_maps.append({"xt": xt, "wt": wtv})

    res = run_bass_kernel_spmd(nc, in_maps, core_ids=list(range(E)), trace=trace)

    out = np.zeros((T, H), np.float32)
    for e in range(E):
        ytv = np.asarray(res.results[e]["yt"]).astype(np.float32)
        # y[c, m*128+p] = yt[p, m*C+c]
        y = ytv.reshape(128, MT, C).transpose(2, 1, 0).reshape(C, H)
        out[idx[e]] = y[: len(idx[e])]
    return out.reshape(S, B, H), res


def kernel(**inputs) -> np.ndarray:
    out, _ = run(**inputs)
    return out
